# revision 16
# baseline (speedup 1.0000x reference)
"""Trainium2 Bass kernel for a 16-head causal self-attention block.

Reference computation (B=1, S=4096, H=2048, 16 heads x 128 dim, fp32):
    qkv = x @ w_qkv.T            # [S, 6144]
    q, k = rope(q), rope(k)      # half-split rope
    attn = causal_softmax(q k^T / sqrt(128)) @ v
    out  = attn @ w_o.T          # [S, 2048]

Sharding: tensor-parallel over heads.  Each of the 8 cores owns 2 heads:
it computes its slice of the QKV projection (768 rows), attention for its
2 heads, and a partial o_proj ([S, 2048]); the host sums the 8 partials.

Dataflow per core (matmul operands bf16, accumulation/softmax fp32):
  phase 1 (QKV+rope), per s-tile of 512: all 16 x^T chunk tiles are loaded
    up front, then three matmul sweeps (V, Q, K) each followed by their
    PSUM->SBUF epilogue, so the single-buffered PSUM banks of one sweep
    drain while the next sweep computes.  RoPE is fused into the epilogue;
    rotate_half is a constant signed-permutation matmul on the PE.
  phase 2 (attention + o_proj interleaved), per q-tile of 512: loop causal
    k-chunks of 128: scoresT[k,q] -> ACT exp (both heads in one [128,1024]
    instr) -> diagonal-mask multiply (GPSIMD) -> PV matmul, while DVE/GPSIMD
    accumulate softmax denominators elementwise; per q-tile a ones-matmul
    folds the partition sums into broadcast form, fast-reciprocal +
    multiply normalizes attn^T, and o_proj for those 512 rows follows.
"""

import numpy as np

import concourse.bass as bass
import concourse.mybir as mybir
import concourse.tile as tile
from concourse import bacc
from concourse.bass_utils import run_bass_kernel_spmd

F32 = mybir.dt.float32
F32R = mybir.dt.float32r
BF16 = mybir.dt.bfloat16

S = 4096
H = 2048
DH = 128
NH = 16
NCORES = 8
HPC = NH // NCORES          # 2 heads per core
OLOC = HPC * DH             # 256 local o-channels per q/k/v group
P = 128
ST1 = 512                   # phase-1 s-tile width
NHT = H // P                # 16 h-chunks
QT = 512                    # phase-2 q-tile width
NQT = S // QT               # 8 q-tiles
NKC = S // P                # 32 k-chunks
SCALE = 1.0 / float(np.sqrt(np.float32(DH)))

_PROGRAM = None


def _build_body(tc):
    nc = tc.nc

    xT = nc.dram_tensor("xT", [H, S], BF16, kind="ExternalInput").ap()
    wqkvT = nc.dram_tensor("wqkvT", [H, 3 * OLOC], BF16, kind="ExternalInput").ap()
    woT = nc.dram_tensor("woT", [OLOC, H], BF16, kind="ExternalInput").ap()
    rope = nc.dram_tensor("rope", [P, 2, S], F32, kind="ExternalInput").ap()
    swapj = nc.dram_tensor("swapj", [P, P], BF16, kind="ExternalInput").ap()
    onesin = nc.dram_tensor("onesin", [P, P], F32R, kind="ExternalInput").ap()
    masks = nc.dram_tensor("masks", [4, P, 2 * QT], BF16, kind="ExternalInput").ap()
    out = nc.dram_tensor("out", [S, H], F32, kind="ExternalOutput").ap()

    xT_v = xT.rearrange("(t p) s -> p t s", p=P)        # [128, 16, 4096]
    wq_v = wqkvT.rearrange("(t p) o -> p t o", p=P)     # [128, 16, 768]
    woT_v = woT.rearrange("(t p) h -> p t h", p=P)      # [128, 2, 2048]

    with tc.tile_pool(name="resident", bufs=1) as resident:
        # d-major Q^T/K^T: [128 d, head, s]; s-major V: [128 s, k-chunk, 256]
        QT_sb = resident.tile([P, HPC, S], BF16)
        KT_sb = resident.tile([P, HPC, S], BF16)
        V_sb = resident.tile([P, NKC, OLOC], BF16)

        # ---------------- phase 1: QKV projection + rope ----------------
        with (
            tc.tile_pool(name="p1w", bufs=1) as p1w,
            tc.tile_pool(name="p1x", bufs=36) as p1x,
            tc.tile_pool(name="p1tab", bufs=3) as p1tab,
            tc.tile_pool(name="p1tmp", bufs=4) as p1tmp,
            tc.tile_pool(name="p1ps", bufs=1, space="PSUM") as p1ps,
            tc.tile_pool(name="p1rot", bufs=2, space="PSUM") as p1rot,
        ):
            J_sb = p1w.tile([P, P], BF16)
            nc.sync.dma_start(J_sb, swapj)
            wT_sb = p1w.tile([P, NHT, 3 * OLOC], BF16)
            for ht in range(NHT):
                nc.sync.dma_start(wT_sb[:, ht, :], wq_v[:, ht, :])

            def rope_block(blk, dst, cos, sin):
                t1 = p1tmp.tile([P, ST1], F32, tag="t1", name="t1")
                t2 = p1tmp.tile([P, ST1], BF16, tag="t2", name="t2")
                nc.vector.tensor_mul(t1, blk, cos)
                nc.vector.tensor_mul(t2, blk, sin)
                rot = p1rot.tile([P, ST1], F32, tag="rot", name="rot")
                nc.tensor.matmul(rot, lhsT=J_sb, rhs=t2, start=True, stop=True)
                nc.vector.tensor_add(dst, t1, rot)

            for st in range(S // ST1):
                s0 = st * ST1
                tab = p1tab.tile([P, 2, ST1], F32, tag="tab")
                nc.sync.dma_start(tab, rope[:, :, s0:s0 + ST1])
                xts = []
                for ht in range(NHT):
                    xt = p1x.tile([P, ST1], BF16, tag="xt", name=f"xt{st}_{ht}")
                    nc.sync.dma_start(xt, xT_v[:, ht, s0:s0 + ST1])
                    xts.append(xt)
                cos = tab[:, 0, :]
                sin = tab[:, 1, :]

                # --- V sweep (pv freed quickly by the ACT copies) ---
                # [128, 1024]: s-subs 0,1 share zero-region/bank 0; 2,3 share 1
                pv = p1ps.tile([P, 2 * ST1], F32, tag="pv", name=f"pv{st}")
                for ht in range(NHT):
                    for sub in range(ST1 // P):
                        nc.tensor.matmul(
                            pv[:, sub * OLOC:(sub + 1) * OLOC],
                            lhsT=xts[ht][:, sub * P:(sub + 1) * P],
                            rhs=wT_sb[:, ht, 2 * OLOC:3 * OLOC],
                            start=(ht == 0) and sub % 2 == 0,
                            stop=(ht == NHT - 1) and sub % 2 == 1,
                        )
                for sub in range(ST1 // P):
                    nc.scalar.activation(
                        V_sb[:, st * (ST1 // P) + sub, :],
                        pv[:, sub * OLOC:(sub + 1) * OLOC],
                        mybir.ActivationFunctionType.Copy,
                    )

                # --- Q sweep + rope ---
                pq = p1ps.tile([P, 2 * ST1], F32, tag="pq", name=f"pq{st}")
                for ht in range(NHT):
                    for h in range(HPC):
                        nc.tensor.matmul(
                            pq[:, h * ST1:(h + 1) * ST1],
                            lhsT=wT_sb[:, ht, h * P:(h + 1) * P],
                            rhs=xts[ht],
                            start=ht == 0, stop=ht == NHT - 1,
                        )
                for h in range(HPC):
                    rope_block(pq[:, h * ST1:(h + 1) * ST1],
                               QT_sb[:, h, s0:s0 + ST1], cos, sin)

                # --- K sweep + rope ---
                pk = p1ps.tile([P, 2 * ST1], F32, tag="pk", name=f"pk{st}")
                for ht in range(NHT):
                    for h in range(HPC):
                        nc.tensor.matmul(
                            pk[:, h * ST1:(h + 1) * ST1],
                            lhsT=wT_sb[:, ht, OLOC + h * P:OLOC + (h + 1) * P],
                            rhs=xts[ht],
                            start=ht == 0, stop=ht == NHT - 1,
                        )
                for h in range(HPC):
                    rope_block(pk[:, h * ST1:(h + 1) * ST1],
                               KT_sb[:, h, s0:s0 + ST1], cos, sin)

        # ---------------- phase 2: attention + o_proj ----------------
        with tc.tile_pool(name="late", bufs=1) as late:
            A_sb = late.tile([P, HPC, S], BF16)          # normalized attn^T
            ones_sb = late.tile([P, P], F32R)
            nc.sync.dma_start(ones_sb, onesin)
            masks_sb = late.tile([P, 4, 2 * QT], BF16)
            nc.sync.dma_start(masks_sb, masks.rearrange("j p q -> p j q"))
            woT_sb = late.tile([P, HPC, H], BF16)
            nc.sync.dma_start(woT_sb, woT_v)

            with (
                tc.tile_pool(name="p2e", bufs=3) as p2e,
                tc.tile_pool(name="p2acc", bufs=2) as p2acc,
                tc.tile_pool(name="p2rec", bufs=4) as p2rec,
                tc.tile_pool(name="p2st", bufs=4) as p2st,
                tc.tile_pool(name="p2sc", bufs=2, space="PSUM") as p2sc,
                tc.tile_pool(name="p2pv", bufs=1, space="PSUM") as p2pv,
                tc.tile_pool(name="p2po", bufs=2, space="PSUM") as p2po,
            ):
                for t in range(NQT):
                    q0 = t * QT
                    nch = 4 * t + 4
                    pv_ps = [
                        p2pv.tile([P, QT], F32, tag=f"pv{h}", name=f"pv{h}_{t}")
                        for h in range(HPC)
                    ]
                    acc = p2acc.tile([P, 2 * QT], F32R, tag="acc")
                    for c in range(nch):
                        first = c == 0
                        last = c == nch - 1
                        sc = p2sc.tile([P, 2 * QT], F32, tag="sc")
                        for h in range(HPC):
                            nc.tensor.matmul(
                                sc[:, h * QT:(h + 1) * QT],
                                lhsT=KT_sb[:, h, c * P:(c + 1) * P],
                                rhs=QT_sb[:, h, q0:q0 + QT],
                                start=True, stop=True,
                            )
                        e = p2e.tile([P, 2 * QT], BF16, tag="e")
                        nc.scalar.activation(
                            e, sc, mybir.ActivationFunctionType.Exp, scale=SCALE
                        )
                        if c >= 4 * t:
                            j = c - 4 * t
                            nc.gpsimd.tensor_mul(e, e, masks_sb[:, j, :])
                        # softmax denominator accumulation (k lives on
                        # partitions): elementwise sum of exp chunks, folded
                        # across partitions once per q-tile below; spread
                        # over DVE and GPSIMD.
                        if first:
                            nc.vector.tensor_copy(acc, e)
                        elif c % 3 == 2:
                            nc.gpsimd.tensor_add(acc, acc, e)
                        else:
                            nc.vector.tensor_add(acc, acc, e)
                        for h in range(HPC):
                            nc.tensor.matmul(
                                pv_ps[h],
                                lhsT=V_sb[:, c, h * P:(h + 1) * P],
                                rhs=e[:, h * QT:(h + 1) * QT],
                                start=first, stop=last,
                            )
                    # fold partition sums -> broadcast [128, q] per head
                    fold = p2sc.tile([P, 2 * QT], F32, tag="sc", name=f"fold{t}")
                    for h in range(HPC):
                        nc.tensor.matmul(
                            fold[:, h * QT:(h + 1) * QT],
                            lhsT=ones_sb,
                            rhs=acc[:, h * QT:(h + 1) * QT],
                            start=True, stop=True,
                        )
                    for h in range(HPC):
                        rec = p2rec.tile([P, QT], F32, tag="rec")
                        nc.vector.reciprocal_approx_fast(
                            rec, fold[:, h * QT:(h + 1) * QT]
                        )
                        nc.vector.tensor_mul(
                            A_sb[:, h, q0:q0 + QT], pv_ps[h], rec
                        )

                    # o_proj for these 512 rows (bf16)
                    for sub in range(QT // P):
                        i = t * (QT // P) + sub
                        for htile in range(H // QT):
                            po = p2po.tile([P, QT], F32, tag="po")
                            for oc in range(HPC):
                                nc.tensor.matmul(
                                    po,
                                    lhsT=A_sb[:, oc, i * P:(i + 1) * P],
                                    rhs=woT_sb[:, oc, htile * QT:(htile + 1) * QT],
                                    start=(oc == 0), stop=(oc == HPC - 1),
                                )
                            stg = p2st.tile([P, QT], F32, tag="stg")
                            nc.vector.tensor_copy(stg, po)
                            nc.sync.dma_start(
                                out[i * P:(i + 1) * P,
                                    htile * QT:(htile + 1) * QT],
                                stg,
                            )


def build_program():
    """Build + compile the Bass program (same program for all 8 cores)."""
    global _PROGRAM
    if _PROGRAM is not None:
        return _PROGRAM
    nc = bacc.Bacc(
        "TRN2", target_bir_lowering=False, debug=False, enable_asserts=False
    )
    with tile.TileContext(nc) as tc:
        _build_body(tc)
    nc.compile()
    _PROGRAM = nc
    return nc


def make_in_maps(hidden_states, w_qkv, w_o):
    import ml_dtypes

    x = np.asarray(hidden_states, dtype=np.float32).reshape(S, H)
    w = np.asarray(w_qkv, dtype=np.float32)
    wo = np.asarray(w_o, dtype=np.float32)

    xT = np.ascontiguousarray(x.T).astype(ml_dtypes.bfloat16)    # [2048, 4096]

    # rope tables, [128, 2, 4096]: rows 0:64 and 64:128 both hold the
    # [64, S] table so the doubled layout lines up with [real; imag] dims.
    e = np.arange(0, DH, 2, dtype=np.float32) / np.float32(DH)
    inv_freq = (1.0 / np.power(np.float32(10000.0), e)).astype(np.float32)
    t = np.arange(S, dtype=np.float32)
    freqs = np.outer(t, inv_freq).astype(np.float32)     # [S, 64]
    cosT = np.cos(freqs).T                               # [64, S]
    sinT = np.sin(freqs).T
    rope = np.empty((P, 2, S), dtype=np.float32)
    rope[0:64, 0] = cosT
    rope[64:128, 0] = cosT
    rope[0:64, 1] = sinT
    rope[64:128, 1] = sinT

    # signed half-swap permutation: (J.T @ z)[d] = -z[64+d], [64+d] = +z[d]
    swapj = np.zeros((P, P), dtype=ml_dtypes.bfloat16)
    for d in range(64):
        swapj[64 + d, d] = -1.0
        swapj[d, 64 + d] = 1.0

    # diagonal-block masks [4, 128, 1024]: chunk at k0 = q0 + 128j keeps
    # (ki, qi) iff qi >= ki + 128j; tiled twice along q for the 2-head tile.
    ki = np.arange(P)[:, None]
    qi = np.arange(QT)[None, :]
    masks = np.empty((4, P, 2 * QT), dtype=ml_dtypes.bfloat16)
    for j in range(4):
        m = (qi >= ki + 128 * j).astype(ml_dtypes.bfloat16)
        masks[j] = np.concatenate([m, m], axis=1)

    in_maps = []
    for c in range(NCORES):
        r0 = c * OLOC
        w_loc = np.concatenate(
            [
                w[r0:r0 + OLOC],
                w[NH * DH + r0:NH * DH + r0 + OLOC],
                w[2 * NH * DH + r0:2 * NH * DH + r0 + OLOC],
            ],
            axis=0,
        )                                                # [768, 2048]
        wqkvT_c = np.ascontiguousarray(w_loc.T).astype(ml_dtypes.bfloat16)
        woT_c = np.ascontiguousarray(
            wo[:, r0:r0 + OLOC].T
        ).astype(ml_dtypes.bfloat16)                     # [256, 2048]
        in_maps.append(
            {
                "xT": xT,
                "wqkvT": wqkvT_c,
                "woT": woT_c,
                "rope": rope,
                "swapj": swapj,
                "onesin": np.ones((P, P), dtype=np.float32),
                "masks": masks,
            }
        )
    return in_maps


def run_cores(in_maps, trace=False, **kwargs):
    nc = build_program()
    return run_bass_kernel_spmd(
        nc, in_maps, list(range(NCORES)), trace=trace, **kwargs
    )


def kernel(hidden_states, w_qkv, w_o):
    in_maps = make_in_maps(hidden_states, w_qkv, w_o)
    res = run_cores(in_maps)
    acc = res.results[0]["out"].astype(np.float32)
    for c in range(1, NCORES):
        acc = acc + res.results[c]["out"]
    return acc.reshape(1, S, H)


# revision 17
# speedup vs baseline: 1.2286x; 1.2286x over previous
"""Trainium2 Bass kernel for a 16-head causal self-attention block.

Reference computation (B=1, S=4096, H=2048, 16 heads x 128 dim, fp32):
    qkv = x @ w_qkv.T            # [S, 6144]
    q, k = rope(q), rope(k)      # half-split rope
    attn = causal_softmax(q k^T / sqrt(128)) @ v
    out  = attn @ w_o.T          # [S, 2048]

Sharding: tensor-parallel over heads.  Each of the 8 cores owns 2 heads:
it computes its slice of the QKV projection (768 rows), attention for its
2 heads, and a partial o_proj ([S, 2048]); the host sums the 8 partials.

Layouts / dataflow per core:
  phase 1 (QKV+rope):  x^T and w^T streamed in bf16; Q^T,K^T d-major
                       [128d, head, S], V s-major [128s, chunk, 256].
                       RoPE fused into the PSUM->SBUF epilogue; rotate_half
                       is a constant signed-permutation matmul on the PE.
  phase 2 (attention + o_proj, interleaved per q-tile of 512):
                       loop causal k-chunks of 128: scoresT[k,q] (bf16 MM)
                       -> ACT exp for both heads in one [128,1024] instr
                       -> diag-mask mult -> PV matmul (bf16) while DVE
                       accumulates softmax denominators; per q-tile: PE ones-
                       matmul folds the partition sums into broadcast form,
                       fast-reciprocal + multiply normalizes attn^T, then
                       o_proj for those 512 rows runs immediately (fp32r).
Accumulations (PSUM), softmax sums and normalization stay fp32.
"""

import numpy as np

import concourse.bass as bass
import concourse.mybir as mybir
import concourse.tile as tile
from concourse import bacc
from concourse.bass_utils import run_bass_kernel_spmd

F32 = mybir.dt.float32
F32R = mybir.dt.float32r
BF16 = mybir.dt.bfloat16

S = 4096
H = 2048
DH = 128
NH = 16
NCORES = 8
HPC = NH // NCORES          # 2 heads per core
OLOC = HPC * DH             # 256 local o-channels per q/k/v group
P = 128
ST1 = 512                   # phase-1 s-tile width
NHT = H // P                # 16 h-chunks
QT = 512                    # phase-2 q-tile width
NQT = S // QT               # 8 q-tiles
NKC = S // P                # 32 k-chunks
SCALE = 1.0 / float(np.sqrt(np.float32(DH)))

_PROGRAM = None


def _build_body(tc):
    nc = tc.nc

    xT = nc.dram_tensor("xT", [H, S], BF16, kind="ExternalInput").ap()
    wqkvT = nc.dram_tensor("wqkvT", [H, 3 * OLOC], BF16, kind="ExternalInput").ap()
    woT = nc.dram_tensor("woT", [OLOC, H], F32R, kind="ExternalInput").ap()
    rope = nc.dram_tensor("rope", [P, 2, S], F32, kind="ExternalInput").ap()
    swapj = nc.dram_tensor("swapj", [P, P], F32R, kind="ExternalInput").ap()
    onesin = nc.dram_tensor("onesin", [P, P], F32R, kind="ExternalInput").ap()
    masks = nc.dram_tensor("masks", [4, P, 2 * QT], BF16, kind="ExternalInput").ap()
    out = nc.dram_tensor("out", [S, H], F32, kind="ExternalOutput").ap()

    xT_v = xT.rearrange("(t p) s -> p t s", p=P)        # [128, 16, 4096]
    wq_v = wqkvT.rearrange("(t p) o -> p t o", p=P)     # [128, 16, 768]
    woT_v = woT.rearrange("(t p) h -> p t h", p=P)      # [128, 2, 2048]

    with tc.tile_pool(name="resident", bufs=1) as resident:
        # d-major Q^T/K^T: [128 d, head, s]; s-major V: [128 s, k-chunk, 256]
        QT_sb = resident.tile([P, HPC, S], BF16)
        KT_sb = resident.tile([P, HPC, S], BF16)
        V_sb = resident.tile([P, NKC, OLOC], BF16)

        # ---------------- phase 1: QKV projection + rope ----------------
        with (
            tc.tile_pool(name="p1w", bufs=1) as p1w,
            tc.tile_pool(name="p1x", bufs=4) as p1x,
            tc.tile_pool(name="p1tab", bufs=3) as p1tab,
            tc.tile_pool(name="p1tmp", bufs=4) as p1tmp,
            tc.tile_pool(name="p1ps", bufs=1, space="PSUM") as p1ps,
            tc.tile_pool(name="p1rot", bufs=2, space="PSUM") as p1rot,
        ):
            J_sb = p1w.tile([P, P], F32R)
            nc.sync.dma_start(J_sb, swapj)
            wT_sb = p1w.tile([P, NHT, 3 * OLOC], BF16)
            for ht in range(NHT):
                nc.sync.dma_start(wT_sb[:, ht, :], wq_v[:, ht, :])

            for st in range(S // ST1):
                s0 = st * ST1
                tab = p1tab.tile([P, 2, ST1], F32, tag="tab")
                nc.sync.dma_start(tab, rope[:, :, s0:s0 + ST1])

                # [128, 1024]: head h occupies [:, h*512:(h+1)*512] (q/k);
                # for v, s-sub i of 4 occupies [:, i*256:(i+1)*256]
                pq = p1ps.tile([P, 2 * ST1], F32, tag="pq")
                pk = p1ps.tile([P, 2 * ST1], F32, tag="pk")
                pv = p1ps.tile([P, 2 * ST1], F32, tag="pv")

                for ht in range(NHT):
                    first = ht == 0
                    last = ht == NHT - 1
                    xt = p1x.tile([P, ST1], BF16, tag="xt")
                    nc.sync.dma_start(xt, xT_v[:, ht, s0:s0 + ST1])
                    for h in range(HPC):
                        nc.tensor.matmul(
                            pq[:, h * ST1:(h + 1) * ST1],
                            lhsT=wT_sb[:, ht, h * P:(h + 1) * P],
                            rhs=xt,
                            start=first, stop=last,
                        )
                        nc.tensor.matmul(
                            pk[:, h * ST1:(h + 1) * ST1],
                            lhsT=wT_sb[:, ht, OLOC + h * P:OLOC + (h + 1) * P],
                            rhs=xt,
                            start=first, stop=last,
                        )
                    # v: 4 s-subs of 128 share the 2-bank pv tensor; start
                    # only the first group per bank, stop only the last.
                    for sub in range(ST1 // P):
                        nc.tensor.matmul(
                            pv[:, sub * OLOC:(sub + 1) * OLOC],
                            lhsT=xt[:, sub * P:(sub + 1) * P],
                            rhs=wT_sb[:, ht, 2 * OLOC:3 * OLOC],
                            start=first and sub % 2 == 0,
                            stop=last and sub % 2 == 1,
                        )

                # epilogue: rope for q/k.  rotate_half is a constant
                # signed-permutation matmul J on the PE (walrus forbids DVE
                # ops whose operands start at different partitions).
                cos = tab[:, 0, :]
                sin = tab[:, 1, :]
                for psum_t, dst_sb in ((pq, QT_sb), (pk, KT_sb)):
                    for h in range(HPC):
                        blk = psum_t[:, h * ST1:(h + 1) * ST1]
                        dst = dst_sb[:, h, s0:s0 + ST1]
                        t1 = p1tmp.tile([P, ST1], F32, tag="t1")
                        t2 = p1tmp.tile([P, ST1], F32R, tag="t2")
                        nc.vector.tensor_mul(t1, blk, cos)
                        nc.vector.tensor_mul(t2, blk, sin)
                        rot = p1rot.tile([P, ST1], F32, tag="rot")
                        nc.tensor.matmul(
                            rot, lhsT=J_sb, rhs=t2, start=True, stop=True
                        )
                        nc.vector.tensor_add(dst, t1, rot)
                for sub in range(ST1 // P):
                    nc.scalar.activation(
                        V_sb[:, st * (ST1 // P) + sub, :],
                        pv[:, sub * OLOC:(sub + 1) * OLOC],
                        mybir.ActivationFunctionType.Copy,
                    )

        # ---------------- phase 2: attention + o_proj ----------------
        with tc.tile_pool(name="late", bufs=1) as late:
            A_sb = late.tile([P, HPC, S], F32R)          # normalized attn^T
            ones_sb = late.tile([P, P], F32R)
            nc.sync.dma_start(ones_sb, onesin)
            masks_sb = late.tile([P, 4, 2 * QT], BF16)
            nc.sync.dma_start(masks_sb, masks.rearrange("j p q -> p j q"))
            woT_sb = late.tile([P, HPC, H], F32R)
            nc.sync.dma_start(woT_sb, woT_v)

            with (
                tc.tile_pool(name="p2e", bufs=3) as p2e,
                tc.tile_pool(name="p2acc", bufs=2) as p2acc,
                tc.tile_pool(name="p2rec", bufs=4) as p2rec,
                tc.tile_pool(name="p2st", bufs=3) as p2st,
                tc.tile_pool(name="p2sc", bufs=2, space="PSUM") as p2sc,
                tc.tile_pool(name="p2pv", bufs=1, space="PSUM") as p2pv,
                tc.tile_pool(name="p2po", bufs=2, space="PSUM") as p2po,
            ):
                for t in range(NQT):
                    q0 = t * QT
                    nch = 4 * t + 4
                    pv_ps = [
                        p2pv.tile([P, QT], F32, tag=f"pv{h}", name=f"pv{h}_{t}")
                        for h in range(HPC)
                    ]
                    acc = p2acc.tile([P, 2 * QT], F32R, tag="acc")
                    for c in range(nch):
                        first = c == 0
                        last = c == nch - 1
                        sc = p2sc.tile([P, 2 * QT], F32, tag="sc")
                        for h in range(HPC):
                            nc.tensor.matmul(
                                sc[:, h * QT:(h + 1) * QT],
                                lhsT=KT_sb[:, h, c * P:(c + 1) * P],
                                rhs=QT_sb[:, h, q0:q0 + QT],
                                start=True, stop=True,
                            )
                        e = p2e.tile([P, 2 * QT], BF16, tag="e")
                        nc.scalar.activation(
                            e, sc, mybir.ActivationFunctionType.Exp, scale=SCALE
                        )
                        if c >= 4 * t:
                            j = c - 4 * t
                            nc.vector.tensor_mul(e, e, masks_sb[:, j, :])
                        # softmax denominator accumulation (k lives on
                        # partitions): elementwise sum of exp chunks, folded
                        # across partitions once per q-tile below.
                        if first:
                            nc.vector.tensor_copy(acc, e)
                        else:
                            nc.vector.tensor_add(acc, acc, e)
                        for h in range(HPC):
                            nc.tensor.matmul(
                                pv_ps[h],
                                lhsT=V_sb[:, c, h * P:(h + 1) * P],
                                rhs=e[:, h * QT:(h + 1) * QT],
                                start=first, stop=last,
                            )
                    # fold partition sums -> broadcast [128, q] per head
                    fold = p2sc.tile([P, 2 * QT], F32, tag="sc", name=f"fold{t}")
                    for h in range(HPC):
                        nc.tensor.matmul(
                            fold[:, h * QT:(h + 1) * QT],
                            lhsT=ones_sb,
                            rhs=acc[:, h * QT:(h + 1) * QT],
                            start=True, stop=True,
                        )
                    for h in range(HPC):
                        rec = p2rec.tile([P, QT], F32, tag="rec")
                        nc.vector.reciprocal_approx_fast(
                            rec, fold[:, h * QT:(h + 1) * QT]
                        )
                        nc.vector.tensor_mul(
                            A_sb[:, h, q0:q0 + QT], pv_ps[h], rec
                        )

                    # o_proj for these 512 rows (fp32r)
                    for sub in range(QT // P):
                        i = t * (QT // P) + sub
                        for htile in range(H // QT):
                            po = p2po.tile([P, QT], F32, tag="po")
                            for oc in range(HPC):
                                nc.tensor.matmul(
                                    po,
                                    lhsT=A_sb[:, oc, i * P:(i + 1) * P],
                                    rhs=woT_sb[:, oc, htile * QT:(htile + 1) * QT],
                                    start=(oc == 0), stop=(oc == HPC - 1),
                                )
                            stg = p2st.tile([P, QT], F32, tag="stg")
                            if htile % 2 == 0:
                                nc.vector.tensor_copy(stg, po)
                            else:
                                nc.scalar.activation(
                                    stg, po, mybir.ActivationFunctionType.Copy
                                )
                            nc.sync.dma_start(
                                out[i * P:(i + 1) * P,
                                    htile * QT:(htile + 1) * QT],
                                stg,
                            )


def build_program():
    """Build + compile the Bass program (same program for all 8 cores)."""
    global _PROGRAM
    if _PROGRAM is not None:
        return _PROGRAM
    nc = bacc.Bacc(
        "TRN2", target_bir_lowering=False, debug=False, enable_asserts=False
    )
    with tile.TileContext(nc) as tc:
        _build_body(tc)
    nc.compile()
    _PROGRAM = nc
    return nc


def make_in_maps(hidden_states, w_qkv, w_o):
    import ml_dtypes

    x = np.asarray(hidden_states, dtype=np.float32).reshape(S, H)
    w = np.asarray(w_qkv, dtype=np.float32)
    wo = np.asarray(w_o, dtype=np.float32)

    xT = np.ascontiguousarray(x.T).astype(ml_dtypes.bfloat16)    # [2048, 4096]

    # rope tables, [128, 2, 4096]: rows 0:64 and 64:128 both hold the
    # [64, S] table so the doubled layout lines up with [real; imag] dims.
    e = np.arange(0, DH, 2, dtype=np.float32) / np.float32(DH)
    inv_freq = (1.0 / np.power(np.float32(10000.0), e)).astype(np.float32)
    t = np.arange(S, dtype=np.float32)
    freqs = np.outer(t, inv_freq).astype(np.float32)     # [S, 64]
    cosT = np.cos(freqs).T                               # [64, S]
    sinT = np.sin(freqs).T
    rope = np.empty((P, 2, S), dtype=np.float32)
    rope[0:64, 0] = cosT
    rope[64:128, 0] = cosT
    rope[0:64, 1] = sinT
    rope[64:128, 1] = sinT

    # signed half-swap permutation: (J.T @ z)[d] = -z[64+d], [64+d] = +z[d]
    swapj = np.zeros((P, P), dtype=np.float32)
    for d in range(64):
        swapj[64 + d, d] = -1.0
        swapj[d, 64 + d] = 1.0

    # diagonal-block masks [4, 128, 1024]: chunk at k0 = q0 + 128j keeps
    # (ki, qi) iff qi >= ki + 128j; tiled twice along q for the 2-head tile.
    ki = np.arange(P)[:, None]
    qi = np.arange(QT)[None, :]
    masks = np.empty((4, P, 2 * QT), dtype=ml_dtypes.bfloat16)
    for j in range(4):
        m = (qi >= ki + 128 * j).astype(ml_dtypes.bfloat16)
        masks[j] = np.concatenate([m, m], axis=1)

    in_maps = []
    for c in range(NCORES):
        r0 = c * OLOC
        w_loc = np.concatenate(
            [
                w[r0:r0 + OLOC],
                w[NH * DH + r0:NH * DH + r0 + OLOC],
                w[2 * NH * DH + r0:2 * NH * DH + r0 + OLOC],
            ],
            axis=0,
        )                                                # [768, 2048]
        wqkvT_c = np.ascontiguousarray(w_loc.T).astype(ml_dtypes.bfloat16)
        woT_c = np.ascontiguousarray(wo[:, r0:r0 + OLOC].T)  # [256, 2048]
        in_maps.append(
            {
                "xT": xT,
                "wqkvT": wqkvT_c,
                "woT": woT_c,
                "rope": rope,
                "swapj": swapj,
                "onesin": np.ones((P, P), dtype=np.float32),
                "masks": masks,
            }
        )
    return in_maps


def run_cores(in_maps, trace=False, **kwargs):
    nc = build_program()
    return run_bass_kernel_spmd(
        nc, in_maps, list(range(NCORES)), trace=trace, **kwargs
    )


def kernel(hidden_states, w_qkv, w_o):
    in_maps = make_in_maps(hidden_states, w_qkv, w_o)
    res = run_cores(in_maps)
    acc = res.results[0]["out"].astype(np.float32)
    for c in range(1, NCORES):
        acc = acc + res.results[c]["out"]
    return acc.reshape(1, S, H)
